# revision 35
# baseline (speedup 1.0000x reference)
"""Trainium2 Bass kernel for nn_Attention (N=4, S=2048, E=1024, H=16, D=64).

Sharding: (batch n, query-half) -> 8 cores, zero communication.
Core c handles batch n=c//2, queries [qh*1024, (qh+1)*1024) with qh=c%2:
  - full multi-head attention for all 16 heads over its 1024 queries
  - its [1024, 1024] slice of the output projection (out @ W_out.T + b_out)

Device dataflow per core (all matmuls bf16, PSUM f32):
  simT[k,q] = K_h @ Q_h^T      (2 heads row-packed on PE quadrants, C=64 each)
  E = exp(simT / 32)           (ScalarE, PSUM -> SBUF bf16; softmax max-sub
                                skipped: logits/32 are in [-2, 2], exact math)
  attT^T/denom = [V_h | 1]^T-stationary matmul over E  (denom = PSUM row 64)
  normalize via reciprocal + selector-matmul partition broadcast
  proj = attT_pairs^T @ W_out^T + b_out  (C=128 e-tiles = head pairs)

The mask input is all-ones by construction (spec fill=ones); where(mask==0)
is a no-op and is not applied on device.
"""

import os
import sys

sys.path.insert(0, "/opt/trn_rl_repo")

import numpy as np
import ml_dtypes

N_CORES = 8
NB, S, E = 4, 2048, 1024
H, D = 16, 64
QL = S // 2          # queries per core
PAIRS = H // 2       # head pairs
KT = S // 128        # k tiles of 128
QC = QL // 512       # q chunks of 512 per core

BF16 = ml_dtypes.bfloat16

_COMPILED = {}       # graph cache
LAST_EXEC_NS = None


# exp(y/32) ~= (1 + y(a1 + y(a2 + y*a3)))^4, Remez-fit on |y/128| <= 0.55;
# max rel err 2.8e-3 for |y| <= 70 (logits are N(0,8), so ~8.8 sigma)
EXP_A1 = 0.007824334282665428
EXP_A2 = 3.115755366175391e-05
EXP_A3 = 7.670041947550437e-08
# ktile groups routed to the VectorE polynomial exp (of 16); the rest use
# ScalarE's table exp. The scalar engine is only ~15% over-committed vs the
# PE's per-group work, so a small offload suffices; more would couple the
# pipeline to the slower DVE through the 2-buffer sim PSUM rotation.
DVE_KTS = frozenset()


def _register_exp_op():
    """Register a custom DVE op: out = (1 + x(C0 + x(C1 + x*C2)))^4.

    Cubic Horner + two squarings = 8 ALU slices. Lets the Vector engine
    compute exp() tiles in parallel with the Scalar engine's table exp,
    which is otherwise the kernel's bottleneck.
    """
    import concourse.dve_ops as do

    if any(op.name == "EXP_P3SQ_ANT" for op in do.OPS):
        return next(op for op in do.OPS if op.name == "EXP_P3SQ_ANT")
    from concourse.dve_spec import Spec, Src0, C0, C1, C2, One, lower, _has_src1, sq
    from concourse.dve_uop import DveOpSpec
    from concourse.dve_table_gen import dve_ver_for

    spec = Spec(
        body=sq(sq(One + Src0 * (C0 + Src0 * (C1 + Src0 * C2)))),
        reference=lambda in0, in1, s0, s1, imm2: (
            1.0 + in0 * (s0 + in0 * (s1 + in0 * imm2))
        ) ** 4,
    )
    name = "EXP_P3SQ_ANT"
    row = do._CUSTOM_DVE_ROW_BASE + len(do.OPS)
    do._SUB_OPCODE_FOR_NAME[name] = row
    ver = dve_ver_for("TRN2")
    tmp = DveOpSpec(name=name, opcode=row, uops=lower(spec, ver=ver),
                    rd1_en=_has_src1(spec))
    op = do.DveOp(name=name, spec=spec, subdim=False,
                  uops_sha={ver: tmp.sha(ver)})
    do.OPS.append(op)
    return op


def _build_graph():
    import concourse.bass as bass
    import concourse.mybir as mybir
    import concourse.tile as tile
    from concourse import bacc

    exp_op = _register_exp_op()
    from concourse.dve_ops import RECIPROCAL_APPROX_NR as _RECIP_NR

    f32 = mybir.dt.float32
    bf16 = mybir.dt.bfloat16
    Exp = mybir.ActivationFunctionType.Exp

    nc = bacc.Bacc("TRN2", target_bir_lowering=False, debug=False,
                   num_devices=N_CORES)

    qt_d = nc.declare_dram_parameter("qt", [128, PAIRS, QL], bf16, isOutput=False)
    kt_d = nc.declare_dram_parameter("kt", [128, PAIRS, S], bf16, isOutput=False)
    va_d = nc.declare_dram_parameter("va", [128, H, KT, 65], bf16, isOutput=False)
    wt_d = nc.declare_dram_parameter("wt", [128, PAIRS, E], bf16, isOutput=False)
    bias_d = nc.declare_dram_parameter("bias", [128, E], f32, isOutput=False)
    sel_d = nc.declare_dram_parameter("sel", [2, 128], bf16, isOutput=False)
    out_d = nc.declare_dram_parameter("out", [QL, E], f32, isOutput=True)

    with tile.TileContext(nc) as tc:
        with (
            tc.tile_pool(name="const", bufs=1) as const_pool,
            tc.tile_pool(name="epool", bufs=4) as e_pool,
            tc.tile_pool(name="stage", bufs=2) as stage_pool,
            tc.tile_pool(name="outp", bufs=4) as out_pool,
            tc.tile_pool(name="simp", bufs=2, space="PSUM") as sim_pool,
            tc.tile_pool(name="avp", bufs=1, space="PSUM") as av_pool,
            tc.tile_pool(name="pbp", bufs=2, space="PSUM") as pb_pool,
        ):
            # resident inputs, DMA'd in per-pair slices so pair 0 starts fast
            va_sb = const_pool.tile([128, H, KT, 65], bf16)
            qt_sb = const_pool.tile([128, PAIRS, QL], bf16)
            kt_sb = const_pool.tile([128, PAIRS, S], bf16)
            for pr in range(PAIRS):
                if pr == 0:
                    # finer slices so the first sim matmul starts ASAP
                    nc.sync.dma_start(qt_sb[:, 0, 0:512], qt_d[:, 0, 0:512])
                    nc.sync.dma_start(kt_sb[:, 0, 0:256], kt_d[:, 0, 0:256])
                    nc.sync.dma_start(va_sb[:, 0:2, 0:2, :], va_d[:, 0:2, 0:2, :])
                    nc.sync.dma_start(kt_sb[:, 0, 256:1024], kt_d[:, 0, 256:1024])
                    nc.sync.dma_start(va_sb[:, 0:2, 2:8, :], va_d[:, 0:2, 2:8, :])
                    nc.sync.dma_start(qt_sb[:, 0, 512:], qt_d[:, 0, 512:])
                    nc.sync.dma_start(kt_sb[:, 0, 1024:], kt_d[:, 0, 1024:])
                    nc.sync.dma_start(va_sb[:, 0:2, 8:, :], va_d[:, 0:2, 8:, :])
                    continue
                nc.sync.dma_start(qt_sb[:, pr, :], qt_d[:, pr, :])
                nc.sync.dma_start(kt_sb[:, pr, :], kt_d[:, pr, :])
                nc.sync.dma_start(va_sb[:, 2 * pr:2 * pr + 2, :, :],
                                  va_d[:, 2 * pr:2 * pr + 2, :, :])
            sel_sb = const_pool.tile([2, 128], bf16)
            wt_sb = const_pool.tile([128, PAIRS, E], bf16)
            bias_sb = const_pool.tile([128, E], f32)

            # per-qc normalized attention^T (e on partitions, q free), bf16
            attnT = [const_pool.tile([128, PAIRS, 512], bf16, tag=f"attnT{i}",
                                     name=f"attnT{i}") for i in range(QC)]


            # deferred-work queue: one micro-op per ktile iteration is
            # drained into the instruction stream, so neither the PE nor the
            # DVE queue ever gets a multi-microsecond block ahead of the
            # latency-critical exp chain
            import collections
            work = collections.deque()

            def drain(n=1):
                for _ in range(n):
                    if work:
                        work.popleft()()

            def attention_pair(qc, pr, hold_scale=False):
                qs = slice(qc * 512, (qc + 1) * 512)
                h1, h2 = 2 * pr, 2 * pr + 1
                av1 = av_pool.tile([65, 512], f32, tag="av1", name="av1")
                av2 = av_pool.tile([65, 512], f32, tag="av2", name="av2")
                for kt in range(KT):
                    drain(1)
                    ks = slice(kt * 128, (kt + 1) * 128)
                    P = sim_pool.tile([128, 2, 512], f32, tag="sim", name="P")
                    nc.tensor.matmul(
                        P[:, 0, :], kt_sb[0:64, pr, ks], qt_sb[0:64, pr, qs],
                        start=True, stop=True,
                    )
                    nc.tensor.matmul(
                        P[:, 1, :], kt_sb[64:128, pr, ks], qt_sb[64:128, pr, qs],
                        start=True, stop=True, tile_position=(64, 0),
                    )
                    Et = e_pool.tile([128, 2, 512], bf16, tag="E", name="Et")
                    if kt in DVE_KTS:
                        nc.vector._custom_dve(
                            exp_op, out=Et[:], in0=P[:],
                            s0=EXP_A1, s1=EXP_A2, imm2=EXP_A3,
                        )
                    else:
                        nc.scalar.activation(Et[:], P[:], Exp, scale=1.0 / 32.0)
                    nc.tensor.matmul(
                        av1[:], va_sb[:, h1, kt, :], Et[:, 0, :],
                        start=(kt == 0), stop=(kt == KT - 1),
                    )
                    nc.tensor.matmul(
                        av2[:], va_sb[:, h2, kt, :], Et[:, 1, :],
                        start=(kt == 0), stop=(kt == KT - 1),
                    )
                # epilogue: unnormalized attT + denominators out of PSUM.
                # Copies run on the Scalar engine (it has slack; the DVE must
                # stay clear for exp tiles).
                dstage = stage_pool.tile([65, 2, 512], f32, tag="dstage",
                                         name="dstage")
                ostage = stage_pool.tile([64, 512], bf16, tag="ostage",
                                         name="ostage")
                nc.vector.tensor_copy(attnT[qc][0:64, pr, :], av1[0:64, :])
                nc.vector.tensor_copy(dstage[64:65, 0, :], av1[64:65, :])
                nc.vector.tensor_copy(ostage[:], av2[0:64, :])
                nc.vector.tensor_copy(dstage[64:65, 1, :], av2[64:65, :])
                # partition relocations (SBUF->SBUF DMA)
                nc.sync.dma_start(attnT[qc][64:128, pr, :], ostage[:])
                dcol = stage_pool.tile([1, 2, 512], f32, tag="dcol",
                                       name="dcol")
                rtmp = stage_pool.tile([1, 2, 2, 512], f32, tag="rtmp",
                                       name="rtmp")
                rb = stage_pool.tile([1, 2, 512], bf16, tag="rb", name="rb")
                nc.sync.dma_start(dcol[:], dstage[64:65, :, :])
                # normalization, deferred op-by-op through the work queue
                work.append(lambda: nc.vector.reciprocal_approx_fast(
                    rtmp[:, 1, :, :], dcol[:]))
                work.append(lambda: nc.vector._custom_dve(
                    _RECIP_NR, out=rtmp[:, 0, :, :], in0=dcol[:],
                    in1=rtmp[:, 1, :, :], s0=2.0))
                work.append(lambda: nc.vector.tensor_copy(rb[:],
                                                          rtmp[:, 0, :, :]))

                def scale():
                    B1 = stage_pool.tile([128, 512], bf16, tag="B1", name="B1")
                    B2 = stage_pool.tile([128, 512], bf16, tag="B2", name="B2")
                    nc.gpsimd.partition_broadcast(B1[:], rb[:, 0, :])
                    nc.gpsimd.partition_broadcast(B2[:], rb[:, 1, :])
                    nc.vector.tensor_mul(
                        attnT[qc][0:64, pr, :], attnT[qc][0:64, pr, :],
                        B1[0:64, :])
                    nc.vector.tensor_mul(
                        attnT[qc][64:128, pr, :], attnT[qc][64:128, pr, :],
                        B2[64:128, :])
                if hold_scale:
                    return scale
                work.append(scale)

            def _proj_mm(qc, idx, pr, st, borrow_sim):
                if st["PP"] is None:
                    if borrow_sim:
                        t = sim_pool.tile([128, 2, 512], f32, tag="sim",
                                          name="PPs")
                        st["PP"] = t[:, 0, :]
                    else:
                        st["PP"] = pb_pool.tile([128, 512], f32, tag="pb",
                                                name="PP")[:]
                lqs = slice((idx // 2) * 128, (idx // 2) * 128 + 128)
                js = slice((idx % 2) * 512, (idx % 2) * 512 + 512)
                nc.tensor.matmul(
                    st["PP"], attnT[qc][:, pr, lqs], wt_sb[:, pr, js],
                    start=(pr == 0), stop=(pr == PAIRS - 1),
                )

            def _proj_epi(qc, idx, st):
                qt_i = 4 * qc + idx // 2
                js = slice((idx % 2) * 512, (idx % 2) * 512 + 512)
                ot = out_pool.tile([128, 512], f32, tag="ot", name="ot")
                nc.vector.tensor_add(ot[:], st["PP"], bias_sb[:, js])
                nc.sync.dma_start(out_d[qt_i * 128:(qt_i + 1) * 128, js], ot[:])

            def queue_proj(qc, lanes=1):
                # `lanes` independent accumulation chains interleaved so tail
                # matmuls pipeline instead of serializing on RAW+LDWEIGHTS
                for g in range(0, 8, lanes):
                    n = min(lanes, 8 - g)
                    sts = [{"PP": None} for _ in range(n)]
                    for pr in range(PAIRS):
                        for ci in range(n):
                            work.append(
                                lambda qc=qc, idx=g + ci, pr=pr, st=sts[ci],
                                       b=(ci >= 2): _proj_mm(qc, idx, pr, st, b)
                            )
                    for ci in range(n):
                        work.append(lambda qc=qc, idx=g + ci, st=sts[ci]:
                                    _proj_epi(qc, idx, st))

            nc.sync.dma_start(sel_sb[:], sel_d[:])
            for pr in range(PAIRS):
                attention_pair(0, pr)
                if pr == 0:
                    # prefetch tail-phase constants once attention is rolling
                    nc.sync.dma_start(bias_sb[:], bias_d[:])
                    nc.sync.dma_start(wt_sb[:], wt_d[:])
            queue_proj(0)
            held_scale = None
            for pr in range(PAIRS):
                sc = attention_pair(1, pr, hold_scale=(pr == PAIRS - 1))
                if sc is not None:
                    held_scale = sc
            # flush remaining deferred norm work (DVE/ACT ops; they overlap
            # the tail matmuls below on their own engines)
            drain(len(work))
            # qc1 projection tail: all 8 PSUM banks are free now, so run all
            # 8 chunks as parallel accumulation chains over pairs 0-6 WHILE
            # pair 7's normalization chain completes, then finish each chunk
            # with its pair-7 matmul.
            pps = []
            pps.append(pb_pool.tile([128, 512], f32, tag="pb", name="tp0")[:])
            pps.append(pb_pool.tile([128, 512], f32, tag="pb", name="tp1")[:])
            for i in range(2):
                t = sim_pool.tile([128, 2, 512], f32, tag="sim", name=f"tps{i}")
                pps.append(t[:, 0, :])
                pps.append(t[:, 1, :])
            pps.append(av_pool.tile([128, 512], f32, tag="av1", name="tpa")[:])

            def tail_mm(idx, pr, PP, start, stop):
                lqs = slice((idx // 2) * 128, (idx // 2) * 128 + 128)
                js = slice((idx % 2) * 512, (idx % 2) * 512 + 512)
                nc.tensor.matmul(PP, attnT[1][:, pr, lqs], wt_sb[:, pr, js],
                                 start=start, stop=stop)

            def tail_epi(idx, PP):
                js = slice((idx % 2) * 512, (idx % 2) * 512 + 512)
                qt_i = 4 + idx // 2
                ot = out_pool.tile([128, 512], f32, tag="ot", name="ot")
                nc.vector.tensor_add(ot[:], PP, bias_sb[:, js])
                nc.sync.dma_start(out_d[qt_i * 128:(qt_i + 1) * 128, js], ot[:])

            for pr in range(PAIRS - 1):
                for idx in range(7):
                    tail_mm(idx, pr, pps[idx], pr == 0, False)
            held_scale()
            for idx in range(7):
                tail_mm(idx, PAIRS - 1, pps[idx], False, True)
                tail_epi(idx, pps[idx])
            pp7 = pb_pool.tile([128, 512], f32, tag="pb", name="tp7")[:]
            for pr in range(PAIRS):
                tail_mm(7, pr, pp7, pr == 0, pr == PAIRS - 1)
            tail_epi(7, pp7)

    nc.compile()
    return nc


def _prep_core_inputs(values, keys, query, W_out, b_out, mask=None):
    """Host-side layout prep: per-core input dicts (bf16, device layouts).

    The mask (all-ones per the spec) is honored exactly anyway: zeroing a
    masked key's row of [V | 1] removes it from both the attention numerator
    and the softmax denominator, which equals where(mask==0, -inf) + softmax.
    """
    wt = np.ascontiguousarray(
        W_out.T.reshape(PAIRS, 128, E).transpose(1, 0, 2)
    ).astype(BF16)
    bias = np.ascontiguousarray(np.tile(b_out[None, :].astype(np.float32), (128, 1)))
    sel = np.zeros((2, 128), dtype=BF16)
    sel[0, 0:64] = 1
    sel[1, 64:128] = 1

    in_maps = []
    for c in range(N_CORES):
        n, qh = c // 2, c % 2
        q_sl = query[n, qh * QL:(qh + 1) * QL]                       # [QL, E]
        qt = np.ascontiguousarray(
            q_sl.reshape(QL, H, D).transpose(1, 2, 0)
            .reshape(PAIRS, 128, QL).transpose(1, 0, 2)
        ).astype(BF16)
        kt = np.ascontiguousarray(
            keys[n].reshape(S, H, D).transpose(1, 2, 0)
            .reshape(PAIRS, 128, S).transpose(1, 0, 2)
        ).astype(BF16)
        v = values[n].reshape(S, H, D)
        va = np.concatenate([v, np.ones((S, H, 1), v.dtype)], axis=2)  # [S,H,65]
        if mask is not None:
            mrow = np.asarray(mask[n]).reshape(-1)
            if mrow.size == S and not np.all(mrow != 0):
                va = va * (mrow != 0)[:, None, None]
        va = np.ascontiguousarray(
            va.transpose(1, 0, 2).reshape(H, KT, 128, 65).transpose(2, 0, 1, 3)
        ).astype(BF16)
        in_maps.append({
            "qt": qt, "kt": kt, "va": va, "wt": wt, "bias": bias, "sel": sel,
        })
    return in_maps


def _install_ntff_hook():
    """Provide antenv.axon_hooks + NTFF profile hook (missing in this image).

    Mirrors trn_boot._ntff_profile_via_ctypes against /opt/axon/libaxon_pjrt.so
    so run_bass_kernel_spmd(trace=True) can capture exec_time_ns.
    """
    import sys as _sys
    import types
    import ctypes
    import contextlib

    if "antenv.axon_hooks" in _sys.modules:
        return
    so_path = "/opt/axon/libaxon_pjrt.so"
    if not os.path.exists(so_path):
        return
    lib = ctypes.CDLL(so_path)
    if not hasattr(lib, "axon_start_nrt_profile"):
        return
    lib.axon_start_nrt_profile.argtypes = [
        ctypes.POINTER(ctypes.c_int64), ctypes.c_size_t]
    lib.axon_start_nrt_profile.restype = ctypes.c_int64
    lib.axon_stop_nrt_profile.argtypes = [ctypes.c_char_p]
    lib.axon_stop_nrt_profile.restype = ctypes.c_int64

    @contextlib.contextmanager
    def _hook(output_dir, device_ids):
        import jax
        jax.devices()
        if device_ids:
            ids = (ctypes.c_int64 * len(device_ids))(*device_ids)
            rc = lib.axon_start_nrt_profile(ids, len(device_ids))
        else:
            rc = lib.axon_start_nrt_profile(None, 0)
        if rc != 0:
            raise RuntimeError(f"axon_start_nrt_profile rc={rc}")
        try:
            yield
        finally:
            n = lib.axon_stop_nrt_profile(str(output_dir).encode())
            print(f"ntff profile: {n} file(s) written to {output_dir}",
                  file=sys.stderr)

    mod = types.ModuleType("antenv.axon_hooks")
    _stash = {"hook": _hook}
    mod.set_axon_ntff_profile_hook = lambda h: _stash.__setitem__("hook", h)
    mod.get_axon_ntff_profile_hook = lambda: _stash["hook"]
    _sys.modules["antenv.axon_hooks"] = mod
    import antenv
    antenv.axon_hooks = mod


def kernel(**inputs):
    global LAST_EXEC_NS
    from concourse.bass_utils import run_bass_kernel_spmd

    values = np.asarray(inputs["values"], dtype=np.float32)
    keys = np.asarray(inputs["keys"], dtype=np.float32)
    query = np.asarray(inputs["query"], dtype=np.float32)
    W_out = np.asarray(inputs["W_out"], dtype=np.float32)
    b_out = np.asarray(inputs["b_out"], dtype=np.float32)
    # inputs["mask"] is all-ones by construction (spec fill=ones): no-op.

    if "nc" not in _COMPILED:
        _COMPILED["nc"] = _build_graph()
    nc = _COMPILED["nc"]

    in_maps = _prep_core_inputs(values, keys, query, W_out, b_out,
                                mask=inputs.get("mask"))
    trace = os.environ.get("KERNEL_TRACE", "0") == "1"
    if trace:
        _install_ntff_hook()
    res = run_bass_kernel_spmd(
        nc, in_maps, core_ids=list(range(N_CORES)), trace=trace,
    )
    LAST_EXEC_NS = res.exec_time_ns

    out = np.empty((NB, S, E), dtype=np.float32)
    for c in range(N_CORES):
        n, qh = c // 2, c % 2
        out[n, qh * QL:(qh + 1) * QL, :] = np.asarray(res.results[c]["out"])
    return out


# revision 36
# speedup vs baseline: 1.0055x; 1.0055x over previous
"""Trainium2 Bass kernel for nn_Attention (N=4, S=2048, E=1024, H=16, D=64).

Sharding: (batch n, query-half) -> 8 cores, zero communication.
Core c handles batch n=c//2, queries [qh*1024, (qh+1)*1024) with qh=c%2:
  - full multi-head attention for all 16 heads over its 1024 queries
  - its [1024, 1024] slice of the output projection (out @ W_out.T + b_out)

Device dataflow per core (all matmuls bf16, PSUM f32):
  simT[k,q] = K_h @ Q_h^T      (2 heads row-packed on PE quadrants, C=64 each)
  E = exp(simT / 32)           (ScalarE, PSUM -> SBUF bf16; softmax max-sub
                                skipped: logits/32 are in [-2, 2], exact math)
  attT^T/denom = [V_h | 1]^T-stationary matmul over E  (denom = PSUM row 64)
  normalize via reciprocal + selector-matmul partition broadcast
  proj = attT_pairs^T @ W_out^T + b_out  (C=128 e-tiles = head pairs)

The mask input is all-ones by construction (spec fill=ones); where(mask==0)
is a no-op and is not applied on device.
"""

import os
import sys

sys.path.insert(0, "/opt/trn_rl_repo")

import numpy as np
import ml_dtypes

N_CORES = 8
NB, S, E = 4, 2048, 1024
H, D = 16, 64
QL = S // 2          # queries per core
PAIRS = H // 2       # head pairs
KT = S // 128        # k tiles of 128
QC = QL // 512       # q chunks of 512 per core

BF16 = ml_dtypes.bfloat16

_COMPILED = {}       # graph cache
LAST_EXEC_NS = None


# exp(y/32) ~= (1 + y(a1 + y(a2 + y*a3)))^4, Remez-fit on |y/128| <= 0.55;
# max rel err 2.8e-3 for |y| <= 70 (logits are N(0,8), so ~8.8 sigma)
EXP_A1 = 0.007824334282665428
EXP_A2 = 3.115755366175391e-05
EXP_A3 = 7.670041947550437e-08
# ktile groups routed to the VectorE polynomial exp (of 16); the rest use
# ScalarE's table exp. The scalar engine is only ~15% over-committed vs the
# PE's per-group work, so a small offload suffices; more would couple the
# pipeline to the slower DVE through the 2-buffer sim PSUM rotation.
DVE_KTS = frozenset()


def _register_exp_op():
    """Register a custom DVE op: out = (1 + x(C0 + x(C1 + x*C2)))^4.

    Cubic Horner + two squarings = 8 ALU slices. Lets the Vector engine
    compute exp() tiles in parallel with the Scalar engine's table exp,
    which is otherwise the kernel's bottleneck.
    """
    import concourse.dve_ops as do

    if any(op.name == "EXP_P3SQ_ANT" for op in do.OPS):
        return next(op for op in do.OPS if op.name == "EXP_P3SQ_ANT")
    from concourse.dve_spec import Spec, Src0, C0, C1, C2, One, lower, _has_src1, sq
    from concourse.dve_uop import DveOpSpec
    from concourse.dve_table_gen import dve_ver_for

    spec = Spec(
        body=sq(sq(One + Src0 * (C0 + Src0 * (C1 + Src0 * C2)))),
        reference=lambda in0, in1, s0, s1, imm2: (
            1.0 + in0 * (s0 + in0 * (s1 + in0 * imm2))
        ) ** 4,
    )
    name = "EXP_P3SQ_ANT"
    row = do._CUSTOM_DVE_ROW_BASE + len(do.OPS)
    do._SUB_OPCODE_FOR_NAME[name] = row
    ver = dve_ver_for("TRN2")
    tmp = DveOpSpec(name=name, opcode=row, uops=lower(spec, ver=ver),
                    rd1_en=_has_src1(spec))
    op = do.DveOp(name=name, spec=spec, subdim=False,
                  uops_sha={ver: tmp.sha(ver)})
    do.OPS.append(op)
    return op


def _build_graph():
    import concourse.bass as bass
    import concourse.mybir as mybir
    import concourse.tile as tile
    from concourse import bacc

    exp_op = _register_exp_op()
    from concourse.dve_ops import RECIPROCAL_APPROX_NR as _RECIP_NR

    f32 = mybir.dt.float32
    bf16 = mybir.dt.bfloat16
    Exp = mybir.ActivationFunctionType.Exp

    nc = bacc.Bacc("TRN2", target_bir_lowering=False, debug=False,
                   num_devices=N_CORES)

    qt_d = nc.declare_dram_parameter("qt", [128, PAIRS, QL], bf16, isOutput=False)
    kt_d = nc.declare_dram_parameter("kt", [128, PAIRS, S], bf16, isOutput=False)
    va_d = nc.declare_dram_parameter("va", [128, H, KT, 65], bf16, isOutput=False)
    wt_d = nc.declare_dram_parameter("wt", [128, PAIRS, E], bf16, isOutput=False)
    bias_d = nc.declare_dram_parameter("bias", [128, E], f32, isOutput=False)
    sel_d = nc.declare_dram_parameter("sel", [2, 128], bf16, isOutput=False)
    out_d = nc.declare_dram_parameter("out", [QL, E], f32, isOutput=True)

    with tile.TileContext(nc) as tc:
        with (
            tc.tile_pool(name="const", bufs=1) as const_pool,
            tc.tile_pool(name="epool", bufs=4) as e_pool,
            tc.tile_pool(name="stage", bufs=2) as stage_pool,
            tc.tile_pool(name="outp", bufs=4) as out_pool,
            tc.tile_pool(name="simp", bufs=2, space="PSUM") as sim_pool,
            tc.tile_pool(name="avp", bufs=1, space="PSUM") as av_pool,
            tc.tile_pool(name="pbp", bufs=2, space="PSUM") as pb_pool,
        ):
            # resident inputs, DMA'd in per-pair slices so pair 0 starts fast
            va_sb = const_pool.tile([128, H, KT, 65], bf16)
            qt_sb = const_pool.tile([128, PAIRS, QL], bf16)
            kt_sb = const_pool.tile([128, PAIRS, S], bf16)
            for pr in range(PAIRS):
                if pr == 0:
                    # finer slices so the first sim matmul starts ASAP
                    nc.sync.dma_start(qt_sb[:, 0, 0:512], qt_d[:, 0, 0:512])
                    nc.sync.dma_start(kt_sb[:, 0, 0:256], kt_d[:, 0, 0:256])
                    nc.sync.dma_start(va_sb[:, 0:2, 0:2, :], va_d[:, 0:2, 0:2, :])
                    nc.sync.dma_start(kt_sb[:, 0, 256:1024], kt_d[:, 0, 256:1024])
                    nc.sync.dma_start(va_sb[:, 0:2, 2:8, :], va_d[:, 0:2, 2:8, :])
                    nc.sync.dma_start(qt_sb[:, 0, 512:], qt_d[:, 0, 512:])
                    nc.sync.dma_start(kt_sb[:, 0, 1024:], kt_d[:, 0, 1024:])
                    nc.sync.dma_start(va_sb[:, 0:2, 8:, :], va_d[:, 0:2, 8:, :])
                    continue
                nc.sync.dma_start(qt_sb[:, pr, :], qt_d[:, pr, :])
                nc.sync.dma_start(kt_sb[:, pr, :], kt_d[:, pr, :])
                nc.sync.dma_start(va_sb[:, 2 * pr:2 * pr + 2, :, :],
                                  va_d[:, 2 * pr:2 * pr + 2, :, :])
            sel_sb = const_pool.tile([2, 128], bf16)
            wt_sb = const_pool.tile([128, PAIRS, E], bf16)
            bias_sb = const_pool.tile([128, E], f32)

            # per-qc normalized attention^T (e on partitions, q free), bf16
            attnT = [const_pool.tile([128, PAIRS, 512], bf16, tag=f"attnT{i}",
                                     name=f"attnT{i}") for i in range(QC)]


            # deferred-work queue: one micro-op per ktile iteration is
            # drained into the instruction stream, so neither the PE nor the
            # DVE queue ever gets a multi-microsecond block ahead of the
            # latency-critical exp chain
            import collections
            work = collections.deque()

            def drain(n=1):
                for _ in range(n):
                    if work:
                        work.popleft()()

            def attention_pair(qc, pr, hold_scale=False):
                qs = slice(qc * 512, (qc + 1) * 512)
                h1, h2 = 2 * pr, 2 * pr + 1
                av1 = av_pool.tile([65, 512], f32, tag="av1", name="av1")
                av2 = av_pool.tile([65, 512], f32, tag="av2", name="av2")
                for kt in range(KT):
                    drain(1)
                    ks = slice(kt * 128, (kt + 1) * 128)
                    P = sim_pool.tile([128, 2, 512], f32, tag="sim", name="P")
                    nc.tensor.matmul(
                        P[:, 0, :], kt_sb[0:64, pr, ks], qt_sb[0:64, pr, qs],
                        start=True, stop=True,
                    )
                    nc.tensor.matmul(
                        P[:, 1, :], kt_sb[64:128, pr, ks], qt_sb[64:128, pr, qs],
                        start=True, stop=True, tile_position=(64, 0),
                    )
                    Et = e_pool.tile([128, 2, 512], bf16, tag="E", name="Et")
                    if kt in DVE_KTS:
                        nc.vector._custom_dve(
                            exp_op, out=Et[:], in0=P[:],
                            s0=EXP_A1, s1=EXP_A2, imm2=EXP_A3,
                        )
                    else:
                        nc.scalar.activation(Et[:], P[:], Exp, scale=1.0 / 32.0)
                    nc.tensor.matmul(
                        av1[:], va_sb[:, h1, kt, :], Et[:, 0, :],
                        start=(kt == 0), stop=(kt == KT - 1),
                    )
                    nc.tensor.matmul(
                        av2[:], va_sb[:, h2, kt, :], Et[:, 1, :],
                        start=(kt == 0), stop=(kt == KT - 1),
                    )
                # epilogue: unnormalized attT + denominators out of PSUM.
                # Copies run on the Scalar engine (it has slack; the DVE must
                # stay clear for exp tiles).
                dstage = stage_pool.tile([65, 2, 512], f32, tag="dstage",
                                         name="dstage")
                ostage = stage_pool.tile([64, 512], bf16, tag="ostage",
                                         name="ostage")
                nc.vector.tensor_copy(attnT[qc][0:64, pr, :], av1[0:64, :])
                nc.vector.tensor_copy(dstage[64:65, 0, :], av1[64:65, :])
                nc.vector.tensor_copy(ostage[:], av2[0:64, :])
                nc.vector.tensor_copy(dstage[64:65, 1, :], av2[64:65, :])
                # partition relocations (SBUF->SBUF DMA)
                nc.sync.dma_start(attnT[qc][64:128, pr, :], ostage[:])
                dcol = stage_pool.tile([2, 512], f32, tag="dcol", name="dcol")
                rtmp = stage_pool.tile([2, 2, 512], f32, tag="rtmp", name="rtmp")
                rb = stage_pool.tile([2, 512], bf16, tag="rb", name="rb")
                nc.sync.dma_start(dcol[:], dstage[64:65, :, :])
                # normalization, deferred op-by-op through the work queue
                work.append(lambda: nc.vector.reciprocal_approx_fast(
                    rtmp[:, 1, :], dcol[:]))
                work.append(lambda: nc.vector._custom_dve(
                    _RECIP_NR, out=rtmp[:, 0, :], in0=dcol[:],
                    in1=rtmp[:, 1, :], s0=2.0))
                work.append(lambda: nc.vector.tensor_copy(rb[:], rtmp[:, 0, :]))

                def scale():
                    if hold_scale:
                        B = av_pool.tile([128, 512], f32, tag="av2", name="Bh")
                    else:
                        B = pb_pool.tile([128, 512], f32, tag="pb", name="B")
                    nc.tensor.matmul(B[:], sel_sb[:], rb[:],
                                     start=True, stop=True)
                    nc.vector.tensor_mul(
                        attnT[qc][:, pr, :], attnT[qc][:, pr, :], B[:])
                if hold_scale:
                    return scale
                work.append(scale)

            def _proj_mm(qc, idx, pr, st, borrow_sim):
                if st["PP"] is None:
                    if borrow_sim:
                        t = sim_pool.tile([128, 2, 512], f32, tag="sim",
                                          name="PPs")
                        st["PP"] = t[:, 0, :]
                    else:
                        st["PP"] = pb_pool.tile([128, 512], f32, tag="pb",
                                                name="PP")[:]
                lqs = slice((idx // 2) * 128, (idx // 2) * 128 + 128)
                js = slice((idx % 2) * 512, (idx % 2) * 512 + 512)
                nc.tensor.matmul(
                    st["PP"], attnT[qc][:, pr, lqs], wt_sb[:, pr, js],
                    start=(pr == 0), stop=(pr == PAIRS - 1),
                )

            def _proj_epi(qc, idx, st):
                qt_i = 4 * qc + idx // 2
                js = slice((idx % 2) * 512, (idx % 2) * 512 + 512)
                ot = out_pool.tile([128, 512], f32, tag="ot", name="ot")
                nc.vector.tensor_add(ot[:], st["PP"], bias_sb[:, js])
                nc.sync.dma_start(out_d[qt_i * 128:(qt_i + 1) * 128, js], ot[:])

            def queue_proj(qc, lanes=1):
                # `lanes` independent accumulation chains interleaved so tail
                # matmuls pipeline instead of serializing on RAW+LDWEIGHTS
                for g in range(0, 8, lanes):
                    n = min(lanes, 8 - g)
                    sts = [{"PP": None} for _ in range(n)]
                    for pr in range(PAIRS):
                        for ci in range(n):
                            work.append(
                                lambda qc=qc, idx=g + ci, pr=pr, st=sts[ci],
                                       b=(ci >= 2): _proj_mm(qc, idx, pr, st, b)
                            )
                    for ci in range(n):
                        work.append(lambda qc=qc, idx=g + ci, st=sts[ci]:
                                    _proj_epi(qc, idx, st))

            nc.sync.dma_start(sel_sb[:], sel_d[:])
            for pr in range(PAIRS):
                attention_pair(0, pr)
                if pr == 0:
                    # prefetch tail-phase constants once attention is rolling
                    nc.sync.dma_start(bias_sb[:], bias_d[:])
                    nc.sync.dma_start(wt_sb[:], wt_d[:])
            queue_proj(0)
            held_scale = None
            for pr in range(PAIRS):
                sc = attention_pair(1, pr, hold_scale=(pr == PAIRS - 1))
                if sc is not None:
                    held_scale = sc
            # flush remaining deferred norm work (DVE/ACT ops; they overlap
            # the tail matmuls below on their own engines)
            drain(len(work))
            # qc1 projection tail: all 8 PSUM banks are free now, so run all
            # 8 chunks as parallel accumulation chains over pairs 0-6 WHILE
            # pair 7's normalization chain completes, then finish each chunk
            # with its pair-7 matmul.
            pps = []
            pps.append(pb_pool.tile([128, 512], f32, tag="pb", name="tp0")[:])
            pps.append(pb_pool.tile([128, 512], f32, tag="pb", name="tp1")[:])
            for i in range(2):
                t = sim_pool.tile([128, 2, 512], f32, tag="sim", name=f"tps{i}")
                pps.append(t[:, 0, :])
                pps.append(t[:, 1, :])
            pps.append(av_pool.tile([128, 512], f32, tag="av1", name="tpa")[:])

            def tail_mm(idx, pr, PP, start, stop):
                lqs = slice((idx // 2) * 128, (idx // 2) * 128 + 128)
                js = slice((idx % 2) * 512, (idx % 2) * 512 + 512)
                nc.tensor.matmul(PP, attnT[1][:, pr, lqs], wt_sb[:, pr, js],
                                 start=start, stop=stop)

            def tail_epi(idx, PP):
                js = slice((idx % 2) * 512, (idx % 2) * 512 + 512)
                qt_i = 4 + idx // 2
                ot = out_pool.tile([128, 512], f32, tag="ot", name="ot")
                nc.vector.tensor_add(ot[:], PP, bias_sb[:, js])
                nc.sync.dma_start(out_d[qt_i * 128:(qt_i + 1) * 128, js], ot[:])

            for pr in range(PAIRS - 1):
                for idx in range(7):
                    tail_mm(idx, pr, pps[idx], pr == 0, False)
            held_scale()
            for idx in range(7):
                tail_mm(idx, PAIRS - 1, pps[idx], False, True)
                tail_epi(idx, pps[idx])
            pp7 = pb_pool.tile([128, 512], f32, tag="pb", name="tp7")[:]
            for pr in range(PAIRS):
                tail_mm(7, pr, pp7, pr == 0, pr == PAIRS - 1)
            tail_epi(7, pp7)

    nc.compile()
    return nc


def _prep_core_inputs(values, keys, query, W_out, b_out, mask=None):
    """Host-side layout prep: per-core input dicts (bf16, device layouts).

    The mask (all-ones per the spec) is honored exactly anyway: zeroing a
    masked key's row of [V | 1] removes it from both the attention numerator
    and the softmax denominator, which equals where(mask==0, -inf) + softmax.
    """
    wt = np.ascontiguousarray(
        W_out.T.reshape(PAIRS, 128, E).transpose(1, 0, 2)
    ).astype(BF16)
    bias = np.ascontiguousarray(np.tile(b_out[None, :].astype(np.float32), (128, 1)))
    sel = np.zeros((2, 128), dtype=BF16)
    sel[0, 0:64] = 1
    sel[1, 64:128] = 1

    in_maps = []
    for c in range(N_CORES):
        n, qh = c // 2, c % 2
        q_sl = query[n, qh * QL:(qh + 1) * QL]                       # [QL, E]
        qt = np.ascontiguousarray(
            q_sl.reshape(QL, H, D).transpose(1, 2, 0)
            .reshape(PAIRS, 128, QL).transpose(1, 0, 2)
        ).astype(BF16)
        kt = np.ascontiguousarray(
            keys[n].reshape(S, H, D).transpose(1, 2, 0)
            .reshape(PAIRS, 128, S).transpose(1, 0, 2)
        ).astype(BF16)
        v = values[n].reshape(S, H, D)
        va = np.concatenate([v, np.ones((S, H, 1), v.dtype)], axis=2)  # [S,H,65]
        if mask is not None:
            mrow = np.asarray(mask[n]).reshape(-1)
            if mrow.size == S and not np.all(mrow != 0):
                va = va * (mrow != 0)[:, None, None]
        va = np.ascontiguousarray(
            va.transpose(1, 0, 2).reshape(H, KT, 128, 65).transpose(2, 0, 1, 3)
        ).astype(BF16)
        in_maps.append({
            "qt": qt, "kt": kt, "va": va, "wt": wt, "bias": bias, "sel": sel,
        })
    return in_maps


def _install_ntff_hook():
    """Provide antenv.axon_hooks + NTFF profile hook (missing in this image).

    Mirrors trn_boot._ntff_profile_via_ctypes against /opt/axon/libaxon_pjrt.so
    so run_bass_kernel_spmd(trace=True) can capture exec_time_ns.
    """
    import sys as _sys
    import types
    import ctypes
    import contextlib

    if "antenv.axon_hooks" in _sys.modules:
        return
    so_path = "/opt/axon/libaxon_pjrt.so"
    if not os.path.exists(so_path):
        return
    lib = ctypes.CDLL(so_path)
    if not hasattr(lib, "axon_start_nrt_profile"):
        return
    lib.axon_start_nrt_profile.argtypes = [
        ctypes.POINTER(ctypes.c_int64), ctypes.c_size_t]
    lib.axon_start_nrt_profile.restype = ctypes.c_int64
    lib.axon_stop_nrt_profile.argtypes = [ctypes.c_char_p]
    lib.axon_stop_nrt_profile.restype = ctypes.c_int64

    @contextlib.contextmanager
    def _hook(output_dir, device_ids):
        import jax
        jax.devices()
        if device_ids:
            ids = (ctypes.c_int64 * len(device_ids))(*device_ids)
            rc = lib.axon_start_nrt_profile(ids, len(device_ids))
        else:
            rc = lib.axon_start_nrt_profile(None, 0)
        if rc != 0:
            raise RuntimeError(f"axon_start_nrt_profile rc={rc}")
        try:
            yield
        finally:
            n = lib.axon_stop_nrt_profile(str(output_dir).encode())
            print(f"ntff profile: {n} file(s) written to {output_dir}",
                  file=sys.stderr)

    mod = types.ModuleType("antenv.axon_hooks")
    _stash = {"hook": _hook}
    mod.set_axon_ntff_profile_hook = lambda h: _stash.__setitem__("hook", h)
    mod.get_axon_ntff_profile_hook = lambda: _stash["hook"]
    _sys.modules["antenv.axon_hooks"] = mod
    import antenv
    antenv.axon_hooks = mod


def kernel(**inputs):
    global LAST_EXEC_NS
    from concourse.bass_utils import run_bass_kernel_spmd

    values = np.asarray(inputs["values"], dtype=np.float32)
    keys = np.asarray(inputs["keys"], dtype=np.float32)
    query = np.asarray(inputs["query"], dtype=np.float32)
    W_out = np.asarray(inputs["W_out"], dtype=np.float32)
    b_out = np.asarray(inputs["b_out"], dtype=np.float32)
    # inputs["mask"] is all-ones by construction (spec fill=ones): no-op.

    if "nc" not in _COMPILED:
        _COMPILED["nc"] = _build_graph()
    nc = _COMPILED["nc"]

    in_maps = _prep_core_inputs(values, keys, query, W_out, b_out,
                                mask=inputs.get("mask"))
    trace = os.environ.get("KERNEL_TRACE", "0") == "1"
    if trace:
        _install_ntff_hook()
    res = run_bass_kernel_spmd(
        nc, in_maps, core_ids=list(range(N_CORES)), trace=trace,
    )
    LAST_EXEC_NS = res.exec_time_ns

    out = np.empty((NB, S, E), dtype=np.float32)
    for c in range(N_CORES):
        n, qh = c // 2, c % 2
        out[n, qh * QL:(qh + 1) * QL, :] = np.asarray(res.results[c]["out"])
    return out
